# revision 34
# baseline (speedup 1.0000x reference)
"""Trainium2 Bass kernel for nn_Enet_81037442941606 (gnn_message_passing).

Computation (reference):
    g   = enc_out[batch_idx, tgt]                      # [N, D] gather
    h0  = batchnorm(g)  (training stats, biased var)   # [N, D]
    h1  = swish(h0 @ wt2_w.T + wt2_b)                  # [N, C]
    out = h1 @ A.T + h1   (A sparse, NNZ entries)      # [N, C]

Strategy (8 NeuronCores, tensor parallel over the class axis):
  * Each core owns a contiguous block of C/8 = 8192 classes: its wt2_w rows,
    its A rows (spmm output rows), and its output columns.
  * Host pre-transposes the W shard so the device reads perfect [d, c] tiles,
    and packs the sparse matrix as per-row-block selection matrices + column
    gather indices (pure data-layout transforms of A).
  * Device: token gather, PE-transpose of the activations, batchnorm stats
    along the free axis (ACT accum_out), in-place normalize; bf16 main matmul
    producing the h1^T shard (f32 resident in SBUF); two chunked bf16
    AllGathers overlapping the matmul tail; then the spmm as indirect
    row-gathers from the gathered h1^T feeding selection-matrix matmuls that
    accumulate in PSUM, fused f32 residual add, transposed output shard out.
  * Host concatenates the 8 output shards and transposes back to [N, C].
"""

import math

import numpy as np
import ml_dtypes

import concourse.bacc as bacc
import concourse.bass as bass
import concourse.mybir as mybir
import concourse.tile as tile
from concourse.bass_utils import run_bass_kernel_spmd
from concourse.masks import make_identity

# Problem sizes (hardcoded per contest rules).
B, S, D, C, N = 32, 128, 1024, 65536, 512
NNZ = 262144
EPS = 1e-5
NCORES = 8
CLOC = C // NCORES          # classes per core = 8192
NB = CLOC // 128            # 64 row-blocks per core
ND = D // 128               # 8 contraction chunks
NT = N // 128               # 4 token tiles
P = 128

EX_DT = mybir.dt.bfloat16   # h1 exchange dtype
EX_NP = ml_dtypes.bfloat16
MM_DT = mybir.dt.bfloat16   # main-matmul operand dtype (W, h0^T)
MM_NP = ml_dtypes.bfloat16

_PROGRAM_CACHE = {}
TRACE = False          # set by test.py to capture an NTFF profile
LAST_RESULTS = None    # BassKernelResults of the last kernel() call


def _build_program(chunks: tuple):
    """Build + compile the SPMD Bass program (identical on all 8 cores).

    chunks[rb] = number of 128-contribution gather/matmul chunks for row
    block rb (same profile on every core; per-core data is padded to it).
    """
    if chunks in _PROGRAM_CACHE:
        return _PROGRAM_CACHE[chunks]
    tot_ch = sum(chunks)

    nc = bacc.Bacc("TRN2", target_bir_lowering=False, debug=False,
                   num_devices=NCORES)
    f32 = mybir.dt.float32
    i32 = mybir.dt.int32

    enc = nc.dram_tensor("enc", [B * S, D], f32, kind="ExternalInput")
    gidx = nc.dram_tensor("gidx", [P, NT], i32, kind="ExternalInput")
    wt = nc.dram_tensor("wt", [NB, P, D], MM_DT, kind="ExternalInput")
    biasv = nc.dram_tensor("biasv", [P, NB], f32, kind="ExternalInput")
    sel = nc.dram_tensor("sel", [P, tot_ch * P], EX_DT, kind="ExternalInput")
    gidxs = nc.dram_tensor("gidxs", [P, tot_ch], i32, kind="ExternalInput")
    outT = nc.dram_tensor("outT", [CLOC, N], f32, kind="ExternalOutput")

    # AllGather is split into NAG chunked collectives over class sub-ranges so
    # the first chunks overlap the tail of the main matmul. Each chunk's
    # output is a strided slice of the single ag_out tensor, keeping one
    # uniform global row index space for the spmm gathers.
    NAG = 2
    CCH = CLOC // NAG
    ag_ins = [nc.dram_tensor(f"ag_in{k}", [CCH, N], EX_DT) for k in range(NAG)]
    ag_out = nc.dram_tensor("ag_out", [C, N], EX_DT, addr_space="Shared")
    # ag_out row space is chunk-major: row = q*(C/NAG) + r*CCH + l for class
    # c = r*CLOC + q*CCH + l. Host remaps gather indices to this layout.
    ag_out_ch = [ag_out[k * (C // NAG):(k + 1) * (C // NAG), :]
                 for k in range(NAG)]
    ag_in_vs = [t.ap().rearrange("(i p) n -> i p n", p=P) for t in ag_ins]
    outT_v = outT.ap().rearrange("(i p) n -> i p n", p=P)

    with tile.TileContext(nc) as tc:
        with (
            tc.tile_pool(name="persist", bufs=1) as persist,
        ):
            # ---------------- Phase A: gather + batchnorm stats + h0^T -----
            h0T = persist.tile([P, ND * N], MM_DT)      # [d%128, (dchunk, n)]
            h1T = persist.tile([P, NB * N], f32)        # [c%128, (ctile, n)]
            bias_t = persist.tile([P, NB], f32)
            gidxs_t = persist.tile([P, tot_ch], i32)
            ident = persist.tile([P, P], f32)
            ones = persist.tile([P, 1], f32)
            mean_s = persist.tile([P, ND], f32)
            rstd_s = persist.tile([P, ND], f32)

            make_identity(nc, ident[:])
            nc.vector.memset(ones[:], 1.0)
            nc.sync.dma_start(out=bias_t[:], in_=biasv[:])
            nc.sync.dma_start(out=gidxs_t[:], in_=gidxs[:])

            gidx_t = persist.tile([P, NT], i32)
            nc.sync.dma_start(out=gidx_t[:], in_=gidx[:])

            with (
                tc.tile_pool(name="phA", bufs=1) as phA,
                tc.tile_pool(name="psA", bufs=4, space="PSUM") as psA,
            ):
                g_tiles = []
                for j in range(NT):
                    g_j = phA.tile([P, D], f32, tag=f"g{j}")
                    nc.gpsimd.indirect_dma_start(
                        out=g_j[:], out_offset=None, in_=enc[:],
                        in_offset=bass.IndirectOffsetOnAxis(
                            ap=gidx_t[:, j:j + 1], axis=0),
                    )
                    g_tiles.append(g_j)

                # Raw transpose g -> h0T (tokens on the free axis)
                for j in range(NT):
                    for i in range(ND):
                        tp = psA.tile([P, P], f32, space="PSUM", tag="tp")
                        nc.tensor.transpose(
                            tp[:], g_tiles[j][:, i * P:(i + 1) * P], ident[:])
                        nc.vector.tensor_copy(
                            out=h0T[:, i * N + j * P: i * N + (j + 1) * P],
                            in_=tp[:])

                # Batch stats along the free (token) axis via ACT accum_out
                sum_s = phA.tile([P, ND], f32, tag="sums")
                sq_s = phA.tile([P, ND], f32, tag="sqs")
                scr = phA.tile([P, N], f32, tag="scr")
                for i in range(ND):
                    nc.scalar.activation(
                        scr[:], h0T[:, i * N:(i + 1) * N],
                        mybir.ActivationFunctionType.Copy,
                        accum_out=sum_s[:, i:i + 1])
                    nc.scalar.activation(
                        scr[:], h0T[:, i * N:(i + 1) * N],
                        mybir.ActivationFunctionType.Square,
                        accum_out=sq_s[:, i:i + 1])

                ex2_s = phA.tile([P, ND], f32, tag="ex2")
                var_s = phA.tile([P, ND], f32, tag="var")
                nc.scalar.mul(mean_s[:], sum_s[:], 1.0 / N)
                nc.scalar.mul(ex2_s[:], sq_s[:], 1.0 / N)
                # var = E[x^2] - mean^2 ; rstd = 1/sqrt(var + eps)
                nc.vector.tensor_tensor(
                    out=var_s[:], in0=mean_s[:], in1=mean_s[:],
                    op=mybir.AluOpType.mult)
                nc.vector.tensor_tensor(
                    out=var_s[:], in0=ex2_s[:], in1=var_s[:],
                    op=mybir.AluOpType.subtract)
                sd_s = phA.tile([P, ND], f32, tag="sd")
                epsb = phA.tile([P, 1], f32, tag="epsb")
                nc.vector.memset(epsb[:], EPS)
                nc.scalar.activation(
                    sd_s[:], var_s[:], mybir.ActivationFunctionType.Sqrt,
                    bias=epsb[:, :1], scale=1.0)
                nc.vector.reciprocal(rstd_s[:], sd_s[:])

                # normalize h0T in place, one wide op per d-chunk
                for i in range(ND):
                    nc.vector.tensor_scalar(
                        out=h0T[:, i * N:(i + 1) * N],
                        in0=h0T[:, i * N:(i + 1) * N],
                        scalar1=mean_s[:, i:i + 1],
                        scalar2=rstd_s[:, i:i + 1],
                        op0=mybir.AluOpType.subtract,
                        op1=mybir.AluOpType.mult,
                    )

            # ---------------- Phase B: h1^T = swish(W h0^T + b) ------------
            # W loads batched 4 c-tiles per DMA (1 MB) to keep PE fed.
            WB = 4
            wt_b = wt.ap().rearrange("(a b) p d -> a b p d", b=WB)
            with (
                tc.tile_pool(name="phB", bufs=3) as phB,
                tc.tile_pool(name="psB", bufs=2, space="PSUM") as psB,
            ):
                for a in range(NB // WB):
                    wt_a = phB.tile([P, WB * D], MM_DT, tag="wt")
                    nc.sync.dma_start(
                        out=wt_a[:].rearrange("p (b d) -> p b d", b=WB),
                        in_=wt_b[a].rearrange("b p d -> p b d"))
                    for bsub in range(WB):
                        i = a * WB + bsub
                        h1ps = psB.tile([P, N], f32, space="PSUM", tag="h1ps")
                        for k in range(ND):
                            nc.tensor.matmul(
                                out=h1ps[:],
                                lhsT=wt_a[:, bsub * D + k * P:
                                          bsub * D + (k + 1) * P],
                                rhs=h0T[:, k * N:(k + 1) * N],
                                start=(k == 0), stop=(k == ND - 1),
                            )
                        nc.scalar.activation(
                            h1T[:, i * N:(i + 1) * N], h1ps[:],
                            mybir.ActivationFunctionType.Silu,
                            bias=bias_t[:, i:i + 1], scale=1.0)
                        h1bf = phB.tile([P, N], EX_DT, tag="h1bf")
                        nc.vector.tensor_copy(
                            out=h1bf[:], in_=h1T[:, i * N:(i + 1) * N])
                        nc.sync.dma_start(
                            out=ag_in_vs[i // (NB // NAG)][i % (NB // NAG)],
                            in_=h1bf[:])

                    # ---- Phase C: chunked AllGathers fire as soon as their
                    # class sub-range of h1^T has been written out.
                    if (a + 1) % (NB // WB // NAG) == 0:
                        k = (a + 1) // (NB // WB // NAG) - 1
                        nc.gpsimd.collective_compute(
                            "AllGather",
                            mybir.AluOpType.bypass,
                            replica_groups=[list(range(NCORES))],
                            ins=[ag_ins[k][:].opt()],
                            outs=[ag_out_ch[k].opt()],
                        )

            # ---------------- Phase D: spmm + residual ---------------------
            with (
                tc.tile_pool(name="phD", bufs=4) as phD,
                tc.tile_pool(name="ctp", bufs=20) as ctp,
                tc.tile_pool(name="psD", bufs=4, space="PSUM") as psD,
            ):
                off = 0
                for rb in range(NB):
                    nch_b = chunks[rb]
                    sel_t = phD.tile([P, max(chunks) * P], EX_DT, tag="sel")
                    nc.sync.dma_start(
                        out=sel_t[:, :nch_b * P],
                        in_=sel[:, off * P:(off + nch_b) * P])
                    acc = psD.tile([P, N], f32, space="PSUM", tag="acc")
                    for ch in range(nch_b):
                        ct = ctp.tile([P, N], EX_DT, tag="ct")
                        nc.gpsimd.indirect_dma_start(
                            out=ct[:], out_offset=None, in_=ag_out[:],
                            in_offset=bass.IndirectOffsetOnAxis(
                                ap=gidxs_t[:, off + ch:off + ch + 1],
                                axis=0),
                        )
                        nc.tensor.matmul(
                            out=acc[:],
                            lhsT=sel_t[:, ch * P:(ch + 1) * P],
                            rhs=ct[:],
                            start=(ch == 0), stop=(ch == nch_b - 1),
                        )
                    o_t = phD.tile([P, N], f32, tag="ot")
                    nc.vector.tensor_tensor(
                        out=o_t[:], in0=acc[:],
                        in1=h1T[:, rb * N:(rb + 1) * N],
                        op=mybir.AluOpType.add)
                    nc.sync.dma_start(out=outT_v[rb], in_=o_t[:])
                    off += nch_b

    nc.compile()
    _PROGRAM_CACHE[chunks] = nc
    return nc


def _prep_host(enc_out, wt2_w, wt2_b, A_values, batch_idx, tgt, A_indices):
    """Shard inputs + restructure the sparse matrix for the device program."""
    enc_flat = np.ascontiguousarray(
        np.asarray(enc_out, dtype=np.float32).reshape(B * S, D))
    flat_idx = (np.asarray(batch_idx, dtype=np.int64) * S
                + np.asarray(tgt, dtype=np.int64)).astype(np.int32)
    gidx_host = np.ascontiguousarray(flat_idx.reshape(NT, P).T)

    wt2_w = np.asarray(wt2_w, dtype=np.float32)
    wt2_b = np.asarray(wt2_b, dtype=np.float32)
    rows_all = np.asarray(A_indices[0], dtype=np.int64)
    cols_all = np.asarray(A_indices[1], dtype=np.int64)
    vals_all = np.asarray(A_values, dtype=np.float32)

    # Per-rank sparse slices + row degrees.
    rank_data = []
    for r in range(NCORES):
        m = (rows_all // CLOC) == r
        rl = (rows_all[m] - r * CLOC).astype(np.int64)
        cc = cols_all[m]
        vv = vals_all[m]
        deg = np.bincount(rl, minlength=CLOC)
        rank_data.append((rl, cc, vv, deg))

    # Pick a global per-block chunk profile: most blocks get 4 chunks
    # (512 contribution slots), NFAT fat blocks (at the end) get 5. Rows are
    # packed into blocks degree-aware so every block fits its capacity.
    # This is a pure relabeling of the class axis within each core: W rows,
    # bias, h1^T tiles, ag_out rows and the final output rows all follow the
    # same permutation (undone on the host at the end).
    max_nnz = max(len(rd[0]) for rd in rank_data)
    base = max(1, max_nnz // (NB * P))   # baseline chunks per block
    nfat = 6                             # fat blocks absorb the remainder
    while True:
        caps = np.full(NB, base * P, dtype=np.int64)
        caps[NB - nfat:] = (base + 1) * P
        perms = []
        ok = True
        for r in range(NCORES):
            deg = rank_data[r][3]
            order = np.argsort(-deg, kind="stable")
            loads = np.zeros(NB, dtype=np.int64)
            cnts = np.zeros(NB, dtype=np.int64)
            assign = np.empty(CLOC, dtype=np.int64)   # row -> bin
            slot = np.empty(CLOC, dtype=np.int64)     # row -> slot in bin
            for row in order:
                d = deg[row]
                score = (loads + d) / caps
                score[cnts >= P] = np.inf
                score[loads + d > caps] = np.inf
                b = int(np.argmin(score))
                if not np.isfinite(score[b]):
                    ok = False
                    break
                assign[row] = b
                slot[row] = cnts[b]
                loads[b] += d
                cnts[b] += 1
            if not ok:
                break
            old2new = assign * P + slot               # old local -> new local
            perms.append(old2new)
        if ok:
            break
        nfat += 4
        if nfat > NB:
            raise RuntimeError("packing failed")
    chunks = tuple(int(caps[rb] // P) for rb in range(NB))
    tot_ch = sum(chunks)
    ch_off = np.zeros(NB, dtype=np.int64)
    ch_off[1:] = np.cumsum(chunks)[:-1]

    NAG = 2
    CCH = CLOC // NAG
    new2old = [np.argsort(p) for p in perms]

    per_rank = []
    for r in range(NCORES):
        rl, cc, vv, _deg = rank_data[r]
        rl_new = perms[r][rl]
        order = np.argsort(rl_new, kind="stable")
        rl_new, cc, vv = rl_new[order], cc[order], vv[order]
        blk = rl_new // P
        counts = np.bincount(blk, minlength=NB)
        starts = np.zeros(NB, dtype=np.int64)
        starts[1:] = np.cumsum(counts)[:-1]
        pos = np.arange(len(rl_new)) - starts[blk]
        ch_idx = pos // P
        p_idx = pos % P
        sel_host = np.zeros((P, tot_ch * P), dtype=EX_NP)
        gidxs_host = np.zeros((P, tot_ch), dtype=np.int32)
        sel_host[p_idx, (ch_off[blk] + ch_idx) * P + (rl_new % P)] = \
            vv.astype(EX_NP)
        # gather row for class c (owner rank rr, old local l): apply rr's
        # permutation, then the chunk-major ag_out layout of the NAG
        # chunked AllGathers: row = q*(C/NAG) + rr*CCH + l'.
        rr = cc // CLOC
        lnew = np.empty(len(cc), dtype=np.int64)
        for r2 in range(NCORES):
            m2 = rr == r2
            lnew[m2] = perms[r2][cc[m2] % CLOC]
        q = lnew // CCH
        ll = lnew % CCH
        ag_row = q * (C // NAG) + rr * CCH + ll
        gidxs_host[p_idx, ch_off[blk] + ch_idx] = ag_row.astype(np.int32)

        rows = slice(r * CLOC, (r + 1) * CLOC)
        wr = wt2_w[rows][new2old[r]]  # [8192, 1024] in permuted order
        wt_host = np.ascontiguousarray(
            wr.reshape(NB, P, ND, P).transpose(0, 3, 2, 1)
        ).reshape(NB, P, D).astype(MM_NP)
        bias_host = np.ascontiguousarray(
            wt2_b[rows][new2old[r]].reshape(NB, P).T)
        per_rank.append({
            "enc": enc_flat,
            "gidx": gidx_host,
            "wt": wt_host,
            "biasv": bias_host,
            "sel": sel_host,
            "gidxs": gidxs_host,
        })
    return per_rank, chunks, new2old


def kernel(**inputs) -> np.ndarray:
    per_rank, chunks, new2old = _prep_host(
        inputs["enc_out"], inputs["wt2_w"], inputs["wt2_b"],
        inputs["A_values"], inputs["batch_idx"], inputs["tgt"],
        inputs["A_indices"])
    nc = _build_program(chunks)
    res = None
    last_exc = None
    for _attempt in range(3):
        try:
            res = run_bass_kernel_spmd(
                nc, per_rank, core_ids=list(range(NCORES)), trace=TRACE)
            break
        except Exception as e:  # transient runtime/collective hiccups
            last_exc = e
    if res is None:
        raise last_exc
    global LAST_RESULTS
    LAST_RESULTS = res
    outT_full = np.empty((C, N), dtype=np.float32)
    for r in range(NCORES):
        outT_full[r * CLOC + new2old[r]] = res.results[r]["outT"]
    return np.ascontiguousarray(outT_full.T)
